# revision 41
# baseline (speedup 1.0000x reference)
"""Multi-head causal attention on 8 Trainium2 NeuronCores.

Sharding: core c handles batch b = c // 2 and head-group g = c % 2
(8 of 16 heads, i.e. 512 of 1024 projection columns).  QKV projections,
attention and the output projection partial run per-core; the two cores
of a batch pair-ReduceScatter their partial outputs (bf16, chunked over
the sequence and overlapped with compute).

Everything on-device is computed in a transposed layout (seq on the
free dim) so no PE transposes are needed anywhere:
  xT [D, L] (host-pre-transposed, bf16) -> qT/kT [512, L] -> S^T [keys, q]
  -> P^T = exp(S^T) (bf16) -> attn^T = (v|ones)^T @ P^T (Z in row 64)
  -> out^T = Wo^T @ attn_norm^T.  Host un-transposes the result.

Pipelining: the emission order interleaves the projections of chunk c+1
and the output projection of chunk c-1 into the attention groups of
chunk c, so the PE never idles long enough for the HAM clock gate to
re-throttle.  S matmuls for the two heads of a pair run concurrently in
disjoint PE row-groups (tile_position).  Softmax normalization uses
DVE reciprocal -> GpSimd partition_broadcast -> DVE multiply (the odd
head's multiply writes directly to partitions 64..127 via the DVE
output crossbar), with no DRAM round trip.
"""

import sys, types

sys.path.insert(0, "/opt/trn_rl_repo")

# antenv.axon_hooks is missing in this image; inject it so trace=True can
# reach the NTFF profiling hook (used by test.py, off by default).
if "antenv.axon_hooks" not in sys.modules:
    _hook_mod = types.ModuleType("antenv.axon_hooks")
    _hook_mod._hook = None
    def _set_hook(h):
        _hook_mod._hook = h
    def _get_hook():
        return _hook_mod._hook
    _hook_mod.set_axon_ntff_profile_hook = _set_hook
    _hook_mod.get_axon_ntff_profile_hook = _get_hook
    sys.modules["antenv.axon_hooks"] = _hook_mod
    try:
        import antenv
        antenv.axon_hooks = _hook_mod
        from trn_agent_boot.trn_boot import _ntff_profile_via_ctypes
        _set_hook(_ntff_profile_via_ctypes("/opt/axon/libaxon_pjrt.so"))
    except Exception:
        pass

import numpy as np
import ml_dtypes
import concourse.bass as bass
import concourse.mybir as mybir
import concourse.tile as tile
from concourse import bacc
from concourse.bass_utils import run_bass_kernel_spmd

B, L, D, H = 4, 2048, 1024, 16
DH = 64
N_CORES = 8
NH = 8          # heads per core
HC = NH * DH    # 512 projection cols per core
QC = 512        # q-chunk
KT = 128        # k-tile
P = 128

F32 = mybir.dt.float32
BF16 = mybir.dt.bfloat16

TRACE = False
LAST_EXEC_NS = None
_NC = None


def build_nc(seq_len=L):
    Ls = seq_len
    NQC = Ls // QC     # 4 sequence chunks
    NKT = Ls // KT     # 16 key tiles
    NDS = D // P       # 8 contraction tiles for projections
    NT = HC // P       # 4 head pairs
    nc = bacc.Bacc()

    xT = nc.declare_dram_parameter("xT", [D, Ls], BF16, isOutput=False)
    wq = nc.declare_dram_parameter("wq", [D, HC], BF16, isOutput=False)
    wk = nc.declare_dram_parameter("wk", [D, HC], BF16, isOutput=False)
    wv = nc.declare_dram_parameter("wv", [D, HC], BF16, isOutput=False)
    wo = nc.declare_dram_parameter("wo", [HC, D], BF16, isOutput=False)
    bq = nc.declare_dram_parameter("bq", [P, HC // P], F32, isOutput=False)
    bk = nc.declare_dram_parameter("bk", [P, HC // P], F32, isOutput=False)
    bv = nc.declare_dram_parameter("bv", [P, HC], F32, isOutput=False)
    bo = nc.declare_dram_parameter("bo", [P, D // P], F32, isOutput=False)
    tri = nc.declare_dram_parameter("tri", [P, P], BF16, isOutput=False)
    # one contiguous [D//2, QC] block per sequence chunk (collective outputs
    # must be contiguous); the host stitches chunks back together
    outTh = nc.declare_dram_parameter("outTh", [Ls // QC, D // 2, QC], BF16, isOutput=True)

    partTs = [nc.dram_tensor(f"partT{c}", [D, QC], BF16) for c in range(NQC)]
    rs_out = nc.dram_tensor("rs_out", [NQC, D // 2, QC], BF16)

    scale = 1.0 / np.sqrt(np.float32(DH))
    IDENT = mybir.ActivationFunctionType.Identity
    EXP = mybir.ActivationFunctionType.Exp

    from contextlib import ExitStack
    with nc.allow_low_precision(reason="bf16 matmul inputs by design"), \
         tile.TileContext(nc) as tc, ExitStack() as ctx:
        out_sems = [ctx.enter_context(nc.semaphore(f"out_sem{c}"))
                    for c in range(NQC)]
        consts = ctx.enter_context(tc.tile_pool(name="consts", bufs=1))
        wpool = ctx.enter_context(tc.tile_pool(name="wpool", bufs=1))
        kvres = ctx.enter_context(tc.tile_pool(name="kvres", bufs=1))
        xtp = ctx.enter_context(tc.tile_pool(name="xtp", bufs=16))
        qtp = ctx.enter_context(tc.tile_pool(name="qtp", bufs=8))
        ptp = ctx.enter_context(tc.tile_pool(name="ptp", bufs=4))
        anp = ctx.enter_context(tc.tile_pool(name="anp", bufs=8))
        otp = ctx.enter_context(tc.tile_pool(name="otp", bufs=3))
        zrp = ctx.enter_context(tc.tile_pool(name="zrp", bufs=2))
        bzp = ctx.enter_context(tc.tile_pool(name="bzp", bufs=2))
        uap = ctx.enter_context(tc.tile_pool(name="uap", bufs=4))
        zvp = ctx.enter_context(tc.tile_pool(name="zvp", bufs=4))
        scratch = ctx.enter_context(tc.tile_pool(name="scratch", bufs=2, space="PSUM"))
        stp = ctx.enter_context(tc.tile_pool(name="stp", bufs=2, space="PSUM"))
        accp = ctx.enter_context(tc.tile_pool(name="accp", bufs=2, space="PSUM"))

        # ---- constants ----
        bq_sb = consts.tile([P, HC // P], F32, tag="bq")
        bk_sb = consts.tile([P, HC // P], F32, tag="bk")
        bv_sb = consts.tile([P, HC], F32, tag="bv")
        bo_sb = consts.tile([P, D // P], F32, tag="bo")
        tri_sb = consts.tile([P, P], BF16, tag="tri")
        nc.sync.dma_start(out=bq_sb, in_=bq[:, :])
        nc.sync.dma_start(out=bk_sb, in_=bk[:, :])
        nc.sync.dma_start(out=bv_sb, in_=bv[:, :])
        nc.sync.dma_start(out=bo_sb, in_=bo[:, :])
        nc.sync.dma_start(out=tri_sb, in_=tri[:, :])

        # ---- weights resident (bf16); spread across engine DMA queues so
        # the sync queue is free for the chunk-0 x tiles (startup latency) ----
        wq_sb = [wpool.tile([P, HC], BF16, tag=f"wq{ds}", name=f"wq{ds}") for ds in range(NDS)]
        wk_sb = [wpool.tile([P, HC], BF16, tag=f"wk{ds}", name=f"wk{ds}") for ds in range(NDS)]
        wv_sb = [wpool.tile([P, HC], BF16, tag=f"wv{ds}", name=f"wv{ds}") for ds in range(NDS)]
        wo_sb = [wpool.tile([P, D], BF16, tag=f"wo{t}", name=f"wo{t}") for t in range(NT)]
        for ds in range(NDS):
            nc.scalar.dma_start(out=wq_sb[ds], in_=wq[ds * P:(ds + 1) * P, :])
        for ds in range(NDS):
            nc.gpsimd.dma_start(out=wk_sb[ds], in_=wk[ds * P:(ds + 1) * P, :])
        for ds in range(NDS):
            nc.scalar.dma_start(out=wv_sb[ds], in_=wv[ds * P:(ds + 1) * P, :])
        for t in range(NT):
            nc.gpsimd.dma_start(out=wo_sb[t], in_=wo[t * P:(t + 1) * P, :])

        # ---- resident kT and v ----
        kT_sb = [kvres.tile([P, Ls], BF16, tag=f"kT{t}", name=f"kT{t}") for t in range(NT)]
        # v: per key-tile [128, NH, 65] bf16; cols 0..63 = v, col 64 = ones
        # (the ones column makes the AV matmul emit softmax Z in row 64)
        v_sb = [kvres.tile([P, NH, 65], BF16, tag=f"v{kt}", name=f"v{kt}") for kt in range(NKT)]
        for kt in range(NKT):
            nc.vector.memset(v_sb[kt], 1.0)

        xT_t = {}      # (ds, s) -> x tile
        qT_t = {}      # (t, s)  -> q tile
        attn_by_chunk = {}

        # ---------- emission units ----------
        def proj_units(s):
            units = []

            def load_x():
                for ds in range(NDS):
                    xt = xtp.tile([P, QC], BF16, tag="xT")
                    nc.sync.dma_start(
                        out=xt, in_=xT[ds * P:(ds + 1) * P, s * QC:(s + 1) * QC])
                    xT_t[(ds, s)] = xt
            units.append(load_x)

            def q_unit(t):
                def f():
                    pq = scratch.tile([P, QC], F32, tag="pacc")
                    for ds in range(NDS):
                        nc.tensor.matmul(
                            pq, wq_sb[ds][:, t * P:(t + 1) * P], xT_t[(ds, s)],
                            start=(ds == 0), stop=(ds == NDS - 1))
                    qt = qtp.tile([P, QC], BF16, tag="qT")
                    nc.vector.tensor_scalar_add(qt, pq, bq_sb[:, t:t + 1])
                    qT_t[(t, s)] = qt
                return f

            def k_unit(t):
                def f():
                    pk = scratch.tile([P, QC], F32, tag="pacc")
                    for ds in range(NDS):
                        nc.tensor.matmul(
                            pk, wk_sb[ds][:, t * P:(t + 1) * P], xT_t[(ds, s)],
                            start=(ds == 0), stop=(ds == NDS - 1))
                    nc.vector.tensor_scalar_add(
                        kT_sb[t][:, s * QC:(s + 1) * QC], pk, bk_sb[:, t:t + 1])
                return f

            def v_unit(sub):
                def f():
                    kt = s * (QC // P) + sub
                    pv = scratch.tile([P, HC], F32, tag="pacc")
                    for ds in range(NDS):
                        nc.tensor.matmul(
                            pv, xT_t[(ds, s)][:, sub * P:(sub + 1) * P], wv_sb[ds],
                            start=(ds == 0), stop=(ds == NDS - 1))
                    nc.vector.tensor_add(
                        v_sb[kt][:, :, 0:64],
                        pv[:].rearrange("p (h d) -> p h d", h=NH),
                        bv_sb[:].rearrange("p (h d) -> p h d", h=NH))
                return f

            for t in range(NT):
                units.append(q_unit(t))
            for t in range(NT):
                units.append(k_unit(t))
            for sub in range(QC // P):
                units.append(v_unit(sub))
            return units

        def outproj_units(oc):
            an_c = attn_by_chunk.pop(oc)
            units = []

            def o_unit(o):
                def f():
                    po = scratch.tile([P, QC], F32, tag="pacc")
                    for t in range(NT):
                        nc.tensor.matmul(
                            po, wo_sb[t][:, o * P:(o + 1) * P], an_c[t],
                            start=(t == 0), stop=(t == NT - 1))
                    ot = otp.tile([P, QC], BF16, tag="ot")
                    nc.vector.tensor_scalar_add(ot, po, bo_sb[:, o:o + 1])
                    nc.sync.dma_start(
                        out=partTs[oc][o * P:(o + 1) * P, :], in_=ot)
                return f

            for o in range(D // P):
                units.append(o_unit(o))
            return units

        def emit_rs(oc):
            # input deps (partT writers) and completion->consumer sync are
            # wired by the tile framework
            nc.gpsimd.collective_compute(
                "ReduceScatter", mybir.AluOpType.add,
                replica_groups=[[0, 1], [2, 3], [4, 5], [6, 7]],
                ins=[partTs[oc][:, :]],
                outs=[rs_out[oc]],
            )

        # ---------- deferred softmax normalization ----------
        # AV results (attn rows 0..63, Z in row 64) are copied out of PSUM
        # immediately (releasing the accumulator bank for the next pair);
        # the 1/Z multiply runs a pair later, off the critical path.
        norm_fifo = []

        def do_norm(an_t, ua_e, ua_o, z_e, z_o):
            # fast approximate 1/Z (5x cheaper than exact reciprocal); inputs
            # are partition-0-aligned so the custom-DVE op never does a
            # cross-quadrant read
            zr_e = zrp.tile([1, QC], F32, tag="zr")
            zr_o = zrp.tile([1, QC], F32, tag="zr")
            nc.vector.reciprocal_approx_fast(out=zr_e, in_=z_e[0:1, :])
            nc.vector.reciprocal_approx_fast(out=zr_o, in_=z_o[0:1, :])
            bz_e = bzp.tile([DH, QC], F32, tag="bz")
            bz_o = bzp.tile([DH, QC], F32, tag="bz")
            nc.gpsimd.partition_broadcast(bz_e, zr_e[0:1, :], channels=DH)
            nc.gpsimd.partition_broadcast(bz_o, zr_o[0:1, :], channels=DH)
            # the odd head writes partitions 64..127 directly (DVE crossbar)
            nc.vector.tensor_mul(an_t[0:DH, :], ua_e[0:DH, :], bz_e)
            nc.vector.tensor_mul(an_t[DH:P, :], ua_o[0:DH, :], bz_o)

        # ---------- chunk 0 projections up-front ----------
        for u in proj_units(0):
            u()

        # ---------- main loop: attention(c) with interleaved fillers ----------
        for c in range(NQC):
            njt = min(4 * c + 4, NKT)
            ngrp = njt // 2

            fillers = []
            op_u = outproj_units(c - 1) if c > 0 else []
            pr_u = proj_units(c + 1) if c + 1 < NQC else []
            # front-load the output projection (2 o-units per proj unit) so
            # the chunk c-1 partials finish early, then trigger the collective
            # mid-chunk so it overlaps the rest of the attention compute
            i = j = 0
            while i < len(pr_u) or j < len(op_u):
                if j < len(op_u):
                    fillers.append(op_u[j]); j += 1
                if j < len(op_u):
                    fillers.append(op_u[j]); j += 1
                    if j == len(op_u):
                        fillers.append(lambda oc=c - 1: emit_rs(oc))
                if i < len(pr_u):
                    fillers.append(pr_u[i]); i += 1

            nslots = NT * ngrp
            pumped = 0

            # drain all fillers by ~85% of the chunk so the next chunk's
            # attention never waits on this chunk's q/k/v projections
            drain_by = max(1, (nslots * 85) // 100)

            def pump(slot_done):
                nonlocal pumped
                due = (len(fillers) * slot_done + drain_by - 1) // drain_by
                while pumped < min(due, len(fillers)):
                    fillers[pumped]()
                    pumped += 1

            slot = 0
            attn_n = {}
            for t in range(NT):
                an_t = anp.tile([P, QC], BF16, tag="an")
                acc_e = accp.tile([P, QC], F32, tag="acc")
                acc_o = accp.tile([P, QC], F32, tag="acc")
                qt = qT_t[(t, c)]

                def s_group(g):
                    """S^T for k-tiles 2g, 2g+1, both heads packed in PE rows."""
                    st_e = stp.tile([P, 2 * QC], F32, tag="st")
                    st_o = stp.tile([P, 2 * QC], F32, tag="st")
                    for half in range(2):
                        jj = 2 * g + half
                        for par, st in ((0, st_e), (1, st_o)):
                            nc.tensor.matmul(
                                st[:, half * QC:(half + 1) * QC],
                                kT_sb[t][par * DH:(par + 1) * DH,
                                         jj * KT:(jj + 1) * KT],
                                qt[par * DH:(par + 1) * DH, :],
                                start=True, stop=True, skip_group_check=True)
                    return st_e, st_o

                sts = s_group(0)
                pts = {}
                for g in range(ngrp):
                    st_e, st_o = sts
                    if g + 1 < ngrp:
                        sts = s_group(g + 1)
                    # exp
                    pt_e = ptp.tile([P, 2 * QC], BF16, tag="pt")
                    pt_o = ptp.tile([P, 2 * QC], BF16, tag="pt")
                    nc.scalar.activation(out=pt_e, in_=st_e, func=EXP,
                                         scale=float(scale))
                    nc.scalar.activation(out=pt_o, in_=st_o, func=EXP,
                                         scale=float(scale))
                    # causal mask on diagonal k-tiles: only the [128,128]
                    # block at the diagonal needs masking; columns left of it
                    # are excluded by the narrowed AV matmul below.
                    for half in range(2):
                        jj = 2 * g + half
                        sub = jj - 4 * c
                        if 0 <= sub < 4:
                            lo = half * QC + sub * P
                            for pt in (pt_e, pt_o):
                                nc.vector.tensor_mul(
                                    pt[:, lo:lo + P], pt[:, lo:lo + P], tri_sb)
                    pts[g] = (pt_e, pt_o)
                    # AV for k-tiles of this group (narrowed on diagonal)
                    for half in range(2):
                        jj = 2 * g + half
                        sub = jj - 4 * c
                        lo = sub * P if 0 <= sub < 4 else 0
                        for par, acc in ((0, acc_e), (1, acc_o)):
                            h = 2 * t + par
                            pt = pts[g][par]
                            nc.tensor.matmul(
                                acc[0:65, lo:QC],
                                v_sb[jj][:, h, :],
                                pt[:, half * QC + lo:(half + 1) * QC],
                                start=(jj == 0), stop=(jj == njt - 1),
                                skip_group_check=True)
                    if g >= 2:
                        pts.pop(g - 2, None)
                    slot += 1
                    pump(slot)

                # ---- copy attn+Z out of PSUM (ACT takes one half, DVE the
                # other, in parallel), freeing the accumulator banks ----
                ua_e = uap.tile([DH, QC], BF16, tag="ua")
                ua_o = uap.tile([DH, QC], BF16, tag="ua")
                z_e = zvp.tile([1, QC], F32, tag="zv")
                z_o = zvp.tile([1, QC], F32, tag="zv")
                nc.scalar.copy(out=ua_e, in_=acc_e[0:DH, :])
                nc.vector.tensor_copy(out=ua_o, in_=acc_o[0:DH, :])
                nc.vector.tensor_copy(out=z_e, in_=acc_e[64:65, :])
                nc.vector.tensor_copy(out=z_o, in_=acc_o[64:65, :])
                norm_fifo.append((an_t, ua_e, ua_o, z_e, z_o))
                # eager in the last chunk (its out-projection follows directly)
                keep = 0 if c == NQC - 1 else 1
                while len(norm_fifo) > keep:
                    do_norm(*norm_fifo.pop(0))
                attn_n[t] = an_t

            attn_by_chunk[c] = attn_n
            # drain remaining fillers and pending norms
            pump(nslots)
            assert pumped == len(fillers)
            while norm_fifo:
                do_norm(*norm_fifo.pop(0))
            # stream finished chunks to the output while compute continues
            if c >= 2:
                oc = c - 2
                nc.gpsimd.dma_start(
                    out=outTh[oc], in_=rs_out[oc]).then_inc(out_sems[oc], 16)

        # ---------- tail: output projection + RS of the last chunk ----------
        for u in outproj_units(NQC - 1):
            u()
        emit_rs(NQC - 1)
        # tile tracks the rs_out write->read dependency (collective completion)
        for oc in (NQC - 2, NQC - 1):
            nc.gpsimd.dma_start(
                out=outTh[oc], in_=rs_out[oc]).then_inc(out_sems[oc], 16)
        for oc in range(NQC):
            nc.gpsimd.wait_ge(out_sems[oc], 16)

    nc.compile()
    return nc


def _make_in_maps(x, Wq, bq, Wk, bk, Wv, bv, Wo, bo, mask):
    ref = np.tril(np.ones((L, L), dtype=np.int32))[None, None]
    assert np.array_equal(np.asarray(mask), ref), "mask must be causal"

    # triangular mask for the diagonal [128,128] block: tri[p, f] = 1 if p <= f
    pf = np.arange(P)[None, :] - np.arange(P)[:, None]
    tri = (pf >= 0).astype(ml_dtypes.bfloat16)

    in_maps = []
    for c in range(N_CORES):
        b, g = c // 2, c % 2
        cols = slice(HC * g, HC * g + HC)
        in_maps.append({
            "xT": np.ascontiguousarray(np.asarray(x[b]).T).astype(ml_dtypes.bfloat16),
            "wq": np.ascontiguousarray(np.asarray(Wq)[:, cols]).astype(ml_dtypes.bfloat16),
            "wk": np.ascontiguousarray(np.asarray(Wk)[:, cols]).astype(ml_dtypes.bfloat16),
            "wv": np.ascontiguousarray(np.asarray(Wv)[:, cols]).astype(ml_dtypes.bfloat16),
            "wo": np.ascontiguousarray(np.asarray(Wo)[cols, :]).astype(ml_dtypes.bfloat16),
            "bq": np.ascontiguousarray(np.asarray(bq)[cols].reshape(HC // P, P).T),
            "bk": np.ascontiguousarray(np.asarray(bk)[cols].reshape(HC // P, P).T),
            "bv": np.ascontiguousarray(
                np.broadcast_to(np.asarray(bv)[cols], (P, HC))),
            "bo": np.ascontiguousarray(
                (np.asarray(bo) / 2.0).reshape(D // P, P).T.astype(np.float32)),
            "tri": tri,
        })
    return in_maps


def kernel(x, Wq, bq, Wk, bk, Wv, bv, Wo, bo, mask):
    global _NC, LAST_EXEC_NS
    if _NC is None:
        _NC = build_nc()
    in_maps = _make_in_maps(x, Wq, bq, Wk, bk, Wv, bv, Wo, bo, mask)
    r = run_bass_kernel_spmd(
        _NC, in_maps, core_ids=list(range(N_CORES)), trace=TRACE)
    LAST_EXEC_NS = r.exec_time_ns
    out = np.empty((B, L, D), dtype=np.float32)
    for b in range(B):
        halves = []
        for c in (2 * b, 2 * b + 1):
            arr = np.asarray(r.results[c]["outTh"]).astype(np.float32)
            halves.append(np.concatenate(list(arr), axis=1))  # [D//2, L]
        out[b] = np.concatenate(halves, axis=0).T
    return out


# revision 44
# speedup vs baseline: 1.1774x; 1.1774x over previous
"""Multi-head causal attention on 8 Trainium2 NeuronCores.

Sharding: core c handles batch b = c // 2 and head-group g = c % 2
(8 of 16 heads, i.e. 512 of 1024 projection columns).  QKV projections,
attention and the output projection partial run per-core; the two cores
of a batch pair-ReduceScatter their partial outputs (bf16, chunked over
the sequence and overlapped with compute).

Everything on-device is computed in a transposed layout (seq on the
free dim) so no PE transposes are needed anywhere:
  xT [D, L] (host-pre-transposed, bf16) -> qT/kT [512, L] -> S^T [keys, q]
  -> P^T = exp(S^T) (bf16) -> attn^T = (v|ones)^T @ P^T (Z in row 64)
  -> out^T = Wo^T @ attn_norm^T.  Host un-transposes the result.

Pipelining: the emission order interleaves the projections of chunk c+1
and the output projection of chunk c-1 into the attention groups of
chunk c, so the PE never idles long enough for the HAM clock gate to
re-throttle.  S matmuls for the two heads of a pair run concurrently in
disjoint PE row-groups (tile_position).  Softmax normalization uses
DVE reciprocal -> GpSimd partition_broadcast -> DVE multiply (the odd
head's multiply writes directly to partitions 64..127 via the DVE
output crossbar), with no DRAM round trip.
"""

import sys, types

sys.path.insert(0, "/opt/trn_rl_repo")

# antenv.axon_hooks is missing in this image; inject it so trace=True can
# reach the NTFF profiling hook (used by test.py, off by default).
if "antenv.axon_hooks" not in sys.modules:
    _hook_mod = types.ModuleType("antenv.axon_hooks")
    _hook_mod._hook = None
    def _set_hook(h):
        _hook_mod._hook = h
    def _get_hook():
        return _hook_mod._hook
    _hook_mod.set_axon_ntff_profile_hook = _set_hook
    _hook_mod.get_axon_ntff_profile_hook = _get_hook
    sys.modules["antenv.axon_hooks"] = _hook_mod
    try:
        import antenv
        antenv.axon_hooks = _hook_mod
        from trn_agent_boot.trn_boot import _ntff_profile_via_ctypes
        _set_hook(_ntff_profile_via_ctypes("/opt/axon/libaxon_pjrt.so"))
    except Exception:
        pass

import numpy as np
import ml_dtypes
import concourse.bass as bass
import concourse.mybir as mybir
import concourse.tile as tile
from concourse import bacc
from concourse.bass_utils import run_bass_kernel_spmd

B, L, D, H = 4, 2048, 1024, 16
DH = 64
N_CORES = 8
NH = 8          # heads per core
HC = NH * DH    # 512 projection cols per core
QC = 512        # q-chunk
KT = 128        # k-tile
P = 128

F32 = mybir.dt.float32
BF16 = mybir.dt.bfloat16

TRACE = False
LAST_EXEC_NS = None
_NC = None


def build_nc(seq_len=L):
    Ls = seq_len
    NQC = Ls // QC     # 4 sequence chunks
    NKT = Ls // KT     # 16 key tiles
    NDS = D // P       # 8 contraction tiles for projections
    NT = HC // P       # 4 head pairs
    nc = bacc.Bacc()

    xT = nc.declare_dram_parameter("xT", [D, Ls], BF16, isOutput=False)
    wq = nc.declare_dram_parameter("wq", [D, HC], BF16, isOutput=False)
    wk = nc.declare_dram_parameter("wk", [D, HC], BF16, isOutput=False)
    wv = nc.declare_dram_parameter("wv", [D, HC], BF16, isOutput=False)
    wo = nc.declare_dram_parameter("wo", [HC, D], BF16, isOutput=False)
    bq = nc.declare_dram_parameter("bq", [P, HC // P], F32, isOutput=False)
    bk = nc.declare_dram_parameter("bk", [P, HC // P], F32, isOutput=False)
    bv = nc.declare_dram_parameter("bv", [P, HC], F32, isOutput=False)
    bo = nc.declare_dram_parameter("bo", [P, D // P], F32, isOutput=False)
    tri = nc.declare_dram_parameter("tri", [P, P], BF16, isOutput=False)
    # one contiguous [D//2, QC] block per sequence chunk (collective outputs
    # must be contiguous); the host stitches chunks back together
    outTh = nc.declare_dram_parameter("outTh", [Ls // QC, D // 2, QC], BF16, isOutput=True)

    partTs = [nc.dram_tensor(f"partT{c}", [D, QC], BF16) for c in range(NQC)]
    rs_out = nc.dram_tensor("rs_out", [NQC, D // 2, QC], BF16)

    scale = 1.0 / np.sqrt(np.float32(DH))
    IDENT = mybir.ActivationFunctionType.Identity
    EXP = mybir.ActivationFunctionType.Exp

    from contextlib import ExitStack
    with nc.allow_low_precision(reason="bf16 matmul inputs by design"), \
         tile.TileContext(nc) as tc, ExitStack() as ctx:
        out_sems = [ctx.enter_context(nc.semaphore(f"out_sem{c}"))
                    for c in range(NQC)]
        consts = ctx.enter_context(tc.tile_pool(name="consts", bufs=1))
        wpool = ctx.enter_context(tc.tile_pool(name="wpool", bufs=1))
        kvres = ctx.enter_context(tc.tile_pool(name="kvres", bufs=1))
        xtp = ctx.enter_context(tc.tile_pool(name="xtp", bufs=16))
        qtp = ctx.enter_context(tc.tile_pool(name="qtp", bufs=8))
        ptp = ctx.enter_context(tc.tile_pool(name="ptp", bufs=4))
        anp = ctx.enter_context(tc.tile_pool(name="anp", bufs=8))
        otp = ctx.enter_context(tc.tile_pool(name="otp", bufs=3))
        zrp = ctx.enter_context(tc.tile_pool(name="zrp", bufs=2))
        bzp = ctx.enter_context(tc.tile_pool(name="bzp", bufs=2))
        uap = ctx.enter_context(tc.tile_pool(name="uap", bufs=4))
        zvp = ctx.enter_context(tc.tile_pool(name="zvp", bufs=4))
        scratch = ctx.enter_context(tc.tile_pool(name="scratch", bufs=2, space="PSUM"))
        stp = ctx.enter_context(tc.tile_pool(name="stp", bufs=2, space="PSUM"))
        accp = ctx.enter_context(tc.tile_pool(name="accp", bufs=2, space="PSUM"))

        # ---- constants ----
        bq_sb = consts.tile([P, HC // P], F32, tag="bq")
        bk_sb = consts.tile([P, HC // P], F32, tag="bk")
        bv_sb = consts.tile([P, HC], F32, tag="bv")
        bo_sb = consts.tile([P, D // P], F32, tag="bo")
        tri_sb = consts.tile([P, P], BF16, tag="tri")
        nc.sync.dma_start(out=bq_sb, in_=bq[:, :])
        nc.sync.dma_start(out=bk_sb, in_=bk[:, :])
        nc.sync.dma_start(out=bv_sb, in_=bv[:, :])
        nc.sync.dma_start(out=bo_sb, in_=bo[:, :])
        nc.sync.dma_start(out=tri_sb, in_=tri[:, :])

        # ---- weights resident (bf16); spread across engine DMA queues so
        # the sync queue is free for the chunk-0 x tiles (startup latency) ----
        wq_sb = [wpool.tile([P, HC], BF16, tag=f"wq{ds}", name=f"wq{ds}") for ds in range(NDS)]
        wk_sb = [wpool.tile([P, HC], BF16, tag=f"wk{ds}", name=f"wk{ds}") for ds in range(NDS)]
        wv_sb = [wpool.tile([P, HC], BF16, tag=f"wv{ds}", name=f"wv{ds}") for ds in range(NDS)]
        wo_sb = [wpool.tile([P, D], BF16, tag=f"wo{t}", name=f"wo{t}") for t in range(NT)]
        for ds in range(NDS):
            nc.scalar.dma_start(out=wq_sb[ds], in_=wq[ds * P:(ds + 1) * P, :])
        for ds in range(NDS):
            nc.gpsimd.dma_start(out=wk_sb[ds], in_=wk[ds * P:(ds + 1) * P, :])
        for ds in range(NDS):
            nc.scalar.dma_start(out=wv_sb[ds], in_=wv[ds * P:(ds + 1) * P, :])
        for t in range(NT):
            nc.gpsimd.dma_start(out=wo_sb[t], in_=wo[t * P:(t + 1) * P, :])

        # ---- resident kT and v ----
        kT_sb = [kvres.tile([P, Ls], BF16, tag=f"kT{t}", name=f"kT{t}") for t in range(NT)]
        # v: per key-tile [128, NH, 65] bf16; cols 0..63 = v, col 64 = ones
        # (the ones column makes the AV matmul emit softmax Z in row 64)
        v_sb = [kvres.tile([P, NH, 65], BF16, tag=f"v{kt}", name=f"v{kt}") for kt in range(NKT)]
        for kt in range(NKT):
            nc.vector.memset(v_sb[kt], 1.0)

        xT_t = {}      # (ds, s) -> x tile
        qT_t = {}      # (t, s)  -> q tile
        attn_by_chunk = {}

        # ---------- emission units ----------
        def proj_units(s):
            units = []

            def load_x():
                for ds in range(NDS):
                    xt = xtp.tile([P, QC], BF16, tag="xT")
                    nc.sync.dma_start(
                        out=xt, in_=xT[ds * P:(ds + 1) * P, s * QC:(s + 1) * QC])
                    xT_t[(ds, s)] = xt
            units.append(load_x)

            def q_unit(t):
                def f():
                    pq = scratch.tile([P, QC], F32, tag="pacc")
                    for ds in range(NDS):
                        nc.tensor.matmul(
                            pq, wq_sb[ds][:, t * P:(t + 1) * P], xT_t[(ds, s)],
                            start=(ds == 0), stop=(ds == NDS - 1))
                    qt = qtp.tile([P, QC], BF16, tag="qT")
                    nc.scalar.activation(
                        out=qt, in_=pq, func=IDENT, bias=bq_sb[:, t:t + 1], scale=1.0)
                    qT_t[(t, s)] = qt
                return f

            def k_unit(t):
                def f():
                    pk = scratch.tile([P, QC], F32, tag="pacc")
                    for ds in range(NDS):
                        nc.tensor.matmul(
                            pk, wk_sb[ds][:, t * P:(t + 1) * P], xT_t[(ds, s)],
                            start=(ds == 0), stop=(ds == NDS - 1))
                    nc.scalar.activation(
                        out=kT_sb[t][:, s * QC:(s + 1) * QC], in_=pk,
                        func=IDENT, bias=bk_sb[:, t:t + 1], scale=1.0)
                return f

            def v_unit(sub):
                def f():
                    kt = s * (QC // P) + sub
                    pv = scratch.tile([P, HC], F32, tag="pacc")
                    for ds in range(NDS):
                        nc.tensor.matmul(
                            pv, xT_t[(ds, s)][:, sub * P:(sub + 1) * P], wv_sb[ds],
                            start=(ds == 0), stop=(ds == NDS - 1))
                    nc.vector.tensor_add(
                        v_sb[kt][:, :, 0:64],
                        pv[:].rearrange("p (h d) -> p h d", h=NH),
                        bv_sb[:].rearrange("p (h d) -> p h d", h=NH))
                return f

            for t in range(NT):
                units.append(q_unit(t))
            for t in range(NT):
                units.append(k_unit(t))
            for sub in range(QC // P):
                units.append(v_unit(sub))
            return units

        def outproj_units(oc):
            an_c = attn_by_chunk.pop(oc)
            units = []

            def o_unit(o):
                def f():
                    po = scratch.tile([P, QC], F32, tag="pacc")
                    for t in range(NT):
                        nc.tensor.matmul(
                            po, wo_sb[t][:, o * P:(o + 1) * P], an_c[t],
                            start=(t == 0), stop=(t == NT - 1))
                    ot = otp.tile([P, QC], BF16, tag="ot")
                    nc.scalar.activation(
                        out=ot, in_=po, func=IDENT, bias=bo_sb[:, o:o + 1], scale=1.0)
                    nc.sync.dma_start(
                        out=partTs[oc][o * P:(o + 1) * P, :], in_=ot)
                return f

            for o in range(D // P):
                units.append(o_unit(o))
            return units

        def emit_rs(oc):
            # input deps (partT writers) and completion->consumer sync are
            # wired by the tile framework
            nc.gpsimd.collective_compute(
                "ReduceScatter", mybir.AluOpType.add,
                replica_groups=[[0, 1], [2, 3], [4, 5], [6, 7]],
                ins=[partTs[oc][:, :]],
                outs=[rs_out[oc]],
            )

        # ---------- deferred softmax normalization ----------
        # AV results (attn rows 0..63, Z in row 64) are copied out of PSUM
        # immediately (releasing the accumulator bank for the next pair);
        # the 1/Z multiply runs a pair later, off the critical path.
        norm_fifo = []

        def do_norm(an_t, ua_e, ua_o, z_e, z_o):
            # fast approximate 1/Z (5x cheaper than exact reciprocal); inputs
            # are partition-0-aligned so the custom-DVE op never does a
            # cross-quadrant read
            zr_e = zrp.tile([1, QC], F32, tag="zr")
            zr_o = zrp.tile([1, QC], F32, tag="zr")
            nc.vector.reciprocal_approx_fast(out=zr_e, in_=z_e[0:1, :])
            nc.vector.reciprocal_approx_fast(out=zr_o, in_=z_o[0:1, :])
            bz_e = bzp.tile([DH, QC], F32, tag="bz")
            bz_o = bzp.tile([DH, QC], F32, tag="bz")
            nc.gpsimd.partition_broadcast(bz_e, zr_e[0:1, :], channels=DH)
            nc.gpsimd.partition_broadcast(bz_o, zr_o[0:1, :], channels=DH)
            # the odd head writes partitions 64..127 directly (DVE crossbar)
            nc.vector.tensor_mul(an_t[0:DH, :], ua_e[0:DH, :], bz_e)
            nc.vector.tensor_mul(an_t[DH:P, :], ua_o[0:DH, :], bz_o)

        # ---------- chunk 0 projections up-front ----------
        for u in proj_units(0):
            u()

        # ---------- main loop: attention(c) with interleaved fillers ----------
        for c in range(NQC):
            njt = min(4 * c + 4, NKT)
            ngrp = njt // 2

            fillers = []
            op_u = outproj_units(c - 1) if c > 0 else []
            pr_u = proj_units(c + 1) if c + 1 < NQC else []
            # front-load the output projection (2 o-units per proj unit) so
            # the chunk c-1 partials finish early, then trigger the collective
            # mid-chunk so it overlaps the rest of the attention compute
            i = j = 0
            while i < len(pr_u) or j < len(op_u):
                if j < len(op_u):
                    fillers.append(op_u[j]); j += 1
                if j < len(op_u):
                    fillers.append(op_u[j]); j += 1
                    if j == len(op_u):
                        fillers.append(lambda oc=c - 1: emit_rs(oc))
                if i < len(pr_u):
                    fillers.append(pr_u[i]); i += 1

            nslots = NT * ngrp
            pumped = 0

            # drain all fillers by ~85% of the chunk so the next chunk's
            # attention never waits on this chunk's q/k/v projections
            drain_by = max(1, (nslots * 85) // 100)

            def pump(slot_done):
                nonlocal pumped
                due = (len(fillers) * slot_done + drain_by - 1) // drain_by
                while pumped < min(due, len(fillers)):
                    fillers[pumped]()
                    pumped += 1

            slot = 0
            attn_n = {}
            for t in range(NT):
                an_t = anp.tile([P, QC], BF16, tag="an")
                acc_e = accp.tile([P, QC], F32, tag="acc")
                acc_o = accp.tile([P, QC], F32, tag="acc")
                qt = qT_t[(t, c)]

                def s_group(g):
                    """S^T for k-tiles 2g, 2g+1, both heads packed in PE rows."""
                    st_e = stp.tile([P, 2 * QC], F32, tag="st")
                    st_o = stp.tile([P, 2 * QC], F32, tag="st")
                    for half in range(2):
                        jj = 2 * g + half
                        for par, st in ((0, st_e), (1, st_o)):
                            nc.tensor.matmul(
                                st[:, half * QC:(half + 1) * QC],
                                kT_sb[t][par * DH:(par + 1) * DH,
                                         jj * KT:(jj + 1) * KT],
                                qt[par * DH:(par + 1) * DH, :],
                                start=True, stop=True, skip_group_check=True)
                    return st_e, st_o

                sts = s_group(0)
                pts = {}
                for g in range(ngrp):
                    st_e, st_o = sts
                    if g + 1 < ngrp:
                        sts = s_group(g + 1)
                    # exp
                    pt_e = ptp.tile([P, 2 * QC], BF16, tag="pt")
                    pt_o = ptp.tile([P, 2 * QC], BF16, tag="pt")
                    nc.scalar.activation(out=pt_e, in_=st_e, func=EXP,
                                         scale=float(scale))
                    nc.scalar.activation(out=pt_o, in_=st_o, func=EXP,
                                         scale=float(scale))
                    # causal mask on diagonal k-tiles: only the [128,128]
                    # block at the diagonal needs masking; columns left of it
                    # are excluded by the narrowed AV matmul below.
                    for half in range(2):
                        jj = 2 * g + half
                        sub = jj - 4 * c
                        if 0 <= sub < 4:
                            lo = half * QC + sub * P
                            for pt in (pt_e, pt_o):
                                nc.vector.tensor_mul(
                                    pt[:, lo:lo + P], pt[:, lo:lo + P], tri_sb)
                    pts[g] = (pt_e, pt_o)
                    # AV for k-tiles of this group (narrowed on diagonal)
                    for half in range(2):
                        jj = 2 * g + half
                        sub = jj - 4 * c
                        lo = sub * P if 0 <= sub < 4 else 0
                        for par, acc in ((0, acc_e), (1, acc_o)):
                            h = 2 * t + par
                            pt = pts[g][par]
                            nc.tensor.matmul(
                                acc[0:65, lo:QC],
                                v_sb[jj][:, h, :],
                                pt[:, half * QC + lo:(half + 1) * QC],
                                start=(jj == 0), stop=(jj == njt - 1),
                                skip_group_check=True)
                    if g >= 2:
                        pts.pop(g - 2, None)
                    slot += 1
                    pump(slot)

                # ---- copy attn+Z out of PSUM (ACT takes one half, DVE the
                # other, in parallel), freeing the accumulator banks ----
                ua_e = uap.tile([DH, QC], BF16, tag="ua")
                ua_o = uap.tile([DH, QC], BF16, tag="ua")
                z_e = zvp.tile([1, QC], F32, tag="zv")
                z_o = zvp.tile([1, QC], F32, tag="zv")
                nc.scalar.copy(out=ua_e, in_=acc_e[0:DH, :])
                nc.vector.tensor_copy(out=ua_o, in_=acc_o[0:DH, :])
                nc.vector.tensor_copy(out=z_e, in_=acc_e[64:65, :])
                nc.vector.tensor_copy(out=z_o, in_=acc_o[64:65, :])
                norm_fifo.append((an_t, ua_e, ua_o, z_e, z_o))
                # eager in the last chunk (its out-projection follows directly)
                keep = 0 if c == NQC - 1 else 1
                while len(norm_fifo) > keep:
                    do_norm(*norm_fifo.pop(0))
                attn_n[t] = an_t

            attn_by_chunk[c] = attn_n
            # drain remaining fillers and pending norms
            pump(nslots)
            assert pumped == len(fillers)
            while norm_fifo:
                do_norm(*norm_fifo.pop(0))
            # stream finished chunks to the output while compute continues
            if c >= 2:
                oc = c - 2
                nc.gpsimd.dma_start(
                    out=outTh[oc], in_=rs_out[oc]).then_inc(out_sems[oc], 16)

        # ---------- tail: output projection + RS of the last chunk ----------
        for u in outproj_units(NQC - 1):
            u()
        emit_rs(NQC - 1)
        # tile tracks the rs_out write->read dependency (collective completion)
        for oc in (NQC - 2, NQC - 1):
            nc.gpsimd.dma_start(
                out=outTh[oc], in_=rs_out[oc]).then_inc(out_sems[oc], 16)
        for oc in range(NQC):
            nc.gpsimd.wait_ge(out_sems[oc], 16)

    nc.compile()
    return nc


def _make_in_maps(x, Wq, bq, Wk, bk, Wv, bv, Wo, bo, mask):
    ref = np.tril(np.ones((L, L), dtype=np.int32))[None, None]
    assert np.array_equal(np.asarray(mask), ref), "mask must be causal"

    # triangular mask for the diagonal [128,128] block: tri[p, f] = 1 if p <= f
    pf = np.arange(P)[None, :] - np.arange(P)[:, None]
    tri = (pf >= 0).astype(ml_dtypes.bfloat16)

    in_maps = []
    for c in range(N_CORES):
        b, g = c // 2, c % 2
        cols = slice(HC * g, HC * g + HC)
        in_maps.append({
            "xT": np.ascontiguousarray(np.asarray(x[b]).T).astype(ml_dtypes.bfloat16),
            "wq": np.ascontiguousarray(np.asarray(Wq)[:, cols]).astype(ml_dtypes.bfloat16),
            "wk": np.ascontiguousarray(np.asarray(Wk)[:, cols]).astype(ml_dtypes.bfloat16),
            "wv": np.ascontiguousarray(np.asarray(Wv)[:, cols]).astype(ml_dtypes.bfloat16),
            "wo": np.ascontiguousarray(np.asarray(Wo)[cols, :]).astype(ml_dtypes.bfloat16),
            "bq": np.ascontiguousarray(np.asarray(bq)[cols].reshape(HC // P, P).T),
            "bk": np.ascontiguousarray(np.asarray(bk)[cols].reshape(HC // P, P).T),
            "bv": np.ascontiguousarray(
                np.broadcast_to(np.asarray(bv)[cols], (P, HC))),
            "bo": np.ascontiguousarray(
                (np.asarray(bo) / 2.0).reshape(D // P, P).T.astype(np.float32)),
            "tri": tri,
        })
    return in_maps


def kernel(x, Wq, bq, Wk, bk, Wv, bv, Wo, bo, mask):
    global _NC, LAST_EXEC_NS
    if _NC is None:
        _NC = build_nc()
    in_maps = _make_in_maps(x, Wq, bq, Wk, bk, Wv, bv, Wo, bo, mask)
    r = run_bass_kernel_spmd(
        _NC, in_maps, core_ids=list(range(N_CORES)), trace=TRACE)
    LAST_EXEC_NS = r.exec_time_ns
    out = np.empty((B, L, D), dtype=np.float32)
    for b in range(B):
        halves = []
        for c in (2 * b, 2 * b + 1):
            arr = np.asarray(r.results[c]["outTh"]).astype(np.float32)
            halves.append(np.concatenate(list(arr), axis=1))  # [D//2, L]
        out[b] = np.concatenate(halves, axis=0).T
    return out


# revision 47
# speedup vs baseline: 1.1845x; 1.0060x over previous
"""Multi-head causal attention on 8 Trainium2 NeuronCores.

Sharding: core c handles batch b = c // 2 and head-group g = c % 2
(8 of 16 heads, i.e. 512 of 1024 projection columns).  QKV projections,
attention and the output projection partial run per-core; the two cores
of a batch pair-ReduceScatter their partial outputs (bf16, chunked over
the sequence and overlapped with compute).

Everything on-device is computed in a transposed layout (seq on the
free dim) so no PE transposes are needed anywhere:
  xT [D, L] (host-pre-transposed, bf16) -> qT/kT [512, L] -> S^T [keys, q]
  -> P^T = exp(S^T) (bf16) -> attn^T = (v|ones)^T @ P^T (Z in row 64)
  -> out^T = Wo^T @ attn_norm^T.  Host un-transposes the result.

Pipelining: the emission order interleaves the projections of chunk c+1
and the output projection of chunk c-1 into the attention groups of
chunk c, so the PE never idles long enough for the HAM clock gate to
re-throttle.  S matmuls for the two heads of a pair run concurrently in
disjoint PE row-groups (tile_position).  Softmax normalization uses
DVE reciprocal -> GpSimd partition_broadcast -> DVE multiply (the odd
head's multiply writes directly to partitions 64..127 via the DVE
output crossbar), with no DRAM round trip.
"""

import sys, types

sys.path.insert(0, "/opt/trn_rl_repo")

# antenv.axon_hooks is missing in this image; inject it so trace=True can
# reach the NTFF profiling hook (used by test.py, off by default).
if "antenv.axon_hooks" not in sys.modules:
    _hook_mod = types.ModuleType("antenv.axon_hooks")
    _hook_mod._hook = None
    def _set_hook(h):
        _hook_mod._hook = h
    def _get_hook():
        return _hook_mod._hook
    _hook_mod.set_axon_ntff_profile_hook = _set_hook
    _hook_mod.get_axon_ntff_profile_hook = _get_hook
    sys.modules["antenv.axon_hooks"] = _hook_mod
    try:
        import antenv
        antenv.axon_hooks = _hook_mod
        from trn_agent_boot.trn_boot import _ntff_profile_via_ctypes
        _set_hook(_ntff_profile_via_ctypes("/opt/axon/libaxon_pjrt.so"))
    except Exception:
        pass

import numpy as np
import ml_dtypes
import concourse.bass as bass
import concourse.mybir as mybir
import concourse.tile as tile
from concourse import bacc
from concourse.bass_utils import run_bass_kernel_spmd

B, L, D, H = 4, 2048, 1024, 16
DH = 64
N_CORES = 8
NH = 8          # heads per core
HC = NH * DH    # 512 projection cols per core
QC = 512        # q-chunk
KT = 128        # k-tile
P = 128

F32 = mybir.dt.float32
BF16 = mybir.dt.bfloat16

TRACE = False
LAST_EXEC_NS = None
_NC = None


def build_nc(seq_len=L):
    Ls = seq_len
    NQC = Ls // QC     # 4 sequence chunks
    NKT = Ls // KT     # 16 key tiles
    NDS = D // P       # 8 contraction tiles for projections
    NT = HC // P       # 4 head pairs
    nc = bacc.Bacc()

    xT = nc.declare_dram_parameter("xT", [D, Ls], BF16, isOutput=False)
    wq = nc.declare_dram_parameter("wq", [D, HC], BF16, isOutput=False)
    wk = nc.declare_dram_parameter("wk", [D, HC], BF16, isOutput=False)
    wv = nc.declare_dram_parameter("wv", [D, HC], BF16, isOutput=False)
    wo = nc.declare_dram_parameter("wo", [HC, D], BF16, isOutput=False)
    bq = nc.declare_dram_parameter("bq", [P, HC // P], F32, isOutput=False)
    bk = nc.declare_dram_parameter("bk", [P, HC // P], F32, isOutput=False)
    bv = nc.declare_dram_parameter("bv", [P, HC], F32, isOutput=False)
    bo = nc.declare_dram_parameter("bo", [P, D // P], F32, isOutput=False)
    tri = nc.declare_dram_parameter("tri", [P, P], BF16, isOutput=False)
    # one contiguous [D//2, QC] block per sequence chunk (collective outputs
    # must be contiguous); the host stitches chunks back together
    outTh = nc.declare_dram_parameter("outTh", [Ls // QC, D // 2, QC], BF16, isOutput=True)

    partTs = [nc.dram_tensor(f"partT{c}", [D, QC], BF16) for c in range(NQC)]
    rs_out = nc.dram_tensor("rs_out", [NQC, D // 2, QC], BF16)

    scale = 1.0 / np.sqrt(np.float32(DH))
    IDENT = mybir.ActivationFunctionType.Identity
    EXP = mybir.ActivationFunctionType.Exp

    from contextlib import ExitStack
    with nc.allow_low_precision(reason="bf16 matmul inputs by design"), \
         tile.TileContext(nc) as tc, ExitStack() as ctx:
        out_sems = [ctx.enter_context(nc.semaphore(f"out_sem{c}"))
                    for c in range(NQC)]
        consts = ctx.enter_context(tc.tile_pool(name="consts", bufs=1))
        wpool = ctx.enter_context(tc.tile_pool(name="wpool", bufs=1))
        kvres = ctx.enter_context(tc.tile_pool(name="kvres", bufs=1))
        xtp = ctx.enter_context(tc.tile_pool(name="xtp", bufs=16))
        qtp = ctx.enter_context(tc.tile_pool(name="qtp", bufs=8))
        ptp = ctx.enter_context(tc.tile_pool(name="ptp", bufs=4))
        anp = ctx.enter_context(tc.tile_pool(name="anp", bufs=8))
        otp = ctx.enter_context(tc.tile_pool(name="otp", bufs=3))
        zrp = ctx.enter_context(tc.tile_pool(name="zrp", bufs=2))
        bzp = ctx.enter_context(tc.tile_pool(name="bzp", bufs=2))
        uap = ctx.enter_context(tc.tile_pool(name="uap", bufs=4))
        zvp = ctx.enter_context(tc.tile_pool(name="zvp", bufs=4))
        scratch = ctx.enter_context(tc.tile_pool(name="scratch", bufs=2, space="PSUM"))
        stp = ctx.enter_context(tc.tile_pool(name="stp", bufs=2, space="PSUM"))
        accp = ctx.enter_context(tc.tile_pool(name="accp", bufs=2, space="PSUM"))

        # ---- constants ----
        bq_sb = consts.tile([P, HC // P], F32, tag="bq")
        bk_sb = consts.tile([P, HC // P], F32, tag="bk")
        bv_sb = consts.tile([P, HC], F32, tag="bv")
        bo_sb = consts.tile([P, D // P], F32, tag="bo")
        tri_sb = consts.tile([P, P], BF16, tag="tri")
        nc.sync.dma_start(out=bq_sb, in_=bq[:, :])
        nc.sync.dma_start(out=bk_sb, in_=bk[:, :])
        nc.sync.dma_start(out=bv_sb, in_=bv[:, :])
        nc.sync.dma_start(out=bo_sb, in_=bo[:, :])
        nc.sync.dma_start(out=tri_sb, in_=tri[:, :])

        # ---- weights resident (bf16); spread across engine DMA queues so
        # the sync queue is free for the chunk-0 x tiles (startup latency) ----
        wq_sb = [wpool.tile([P, HC], BF16, tag=f"wq{ds}", name=f"wq{ds}") for ds in range(NDS)]
        wk_sb = [wpool.tile([P, HC], BF16, tag=f"wk{ds}", name=f"wk{ds}") for ds in range(NDS)]
        wv_sb = [wpool.tile([P, HC], BF16, tag=f"wv{ds}", name=f"wv{ds}") for ds in range(NDS)]
        wo_sb = [wpool.tile([P, D], BF16, tag=f"wo{t}", name=f"wo{t}") for t in range(NT)]
        for ds in range(NDS):
            nc.scalar.dma_start(out=wq_sb[ds], in_=wq[ds * P:(ds + 1) * P, :])
        for ds in range(NDS):
            nc.scalar.dma_start(out=wv_sb[ds], in_=wv[ds * P:(ds + 1) * P, :])
        for t in range(NT):
            nc.gpsimd.dma_start(out=wo_sb[t], in_=wo[t * P:(t + 1) * P, :])

        def load_wk():
            # sync queue (fast HW DGE), after the chunk-0 x tiles
            for ds in range(NDS):
                nc.sync.dma_start(out=wk_sb[ds], in_=wk[ds * P:(ds + 1) * P, :])

        # ---- resident kT and v ----
        kT_sb = [kvres.tile([P, Ls], BF16, tag=f"kT{t}", name=f"kT{t}") for t in range(NT)]
        # v: per key-tile [128, NH, 65] bf16; cols 0..63 = v, col 64 = ones
        # (the ones column makes the AV matmul emit softmax Z in row 64)
        v_sb = [kvres.tile([P, NH, 65], BF16, tag=f"v{kt}", name=f"v{kt}") for kt in range(NKT)]
        for kt in range(NKT):
            nc.vector.memset(v_sb[kt], 1.0)

        xT_t = {}      # (ds, s) -> x tile
        qT_t = {}      # (t, s)  -> q tile
        attn_by_chunk = {}

        # ---------- emission units ----------
        def proj_units(s):
            units = []

            def load_x():
                for ds in range(NDS):
                    xt = xtp.tile([P, QC], BF16, tag="xT")
                    nc.sync.dma_start(
                        out=xt, in_=xT[ds * P:(ds + 1) * P, s * QC:(s + 1) * QC])
                    xT_t[(ds, s)] = xt
            units.append(load_x)

            def q_unit(t):
                def f():
                    pq = scratch.tile([P, QC], F32, tag="pacc")
                    for ds in range(NDS):
                        nc.tensor.matmul(
                            pq, wq_sb[ds][:, t * P:(t + 1) * P], xT_t[(ds, s)],
                            start=(ds == 0), stop=(ds == NDS - 1))
                    qt = qtp.tile([P, QC], BF16, tag="qT")
                    nc.scalar.activation(
                        out=qt, in_=pq, func=IDENT, bias=bq_sb[:, t:t + 1], scale=1.0)
                    qT_t[(t, s)] = qt
                return f

            def k_unit(t):
                def f():
                    pk = scratch.tile([P, QC], F32, tag="pacc")
                    for ds in range(NDS):
                        nc.tensor.matmul(
                            pk, wk_sb[ds][:, t * P:(t + 1) * P], xT_t[(ds, s)],
                            start=(ds == 0), stop=(ds == NDS - 1))
                    nc.scalar.activation(
                        out=kT_sb[t][:, s * QC:(s + 1) * QC], in_=pk,
                        func=IDENT, bias=bk_sb[:, t:t + 1], scale=1.0)
                return f

            def v_unit(sub):
                def f():
                    kt = s * (QC // P) + sub
                    pv = scratch.tile([P, HC], F32, tag="pacc")
                    for ds in range(NDS):
                        nc.tensor.matmul(
                            pv, xT_t[(ds, s)][:, sub * P:(sub + 1) * P], wv_sb[ds],
                            start=(ds == 0), stop=(ds == NDS - 1))
                    nc.vector.tensor_add(
                        v_sb[kt][:, :, 0:64],
                        pv[:].rearrange("p (h d) -> p h d", h=NH),
                        bv_sb[:].rearrange("p (h d) -> p h d", h=NH))
                return f

            for t in range(NT):
                units.append(q_unit(t))
            for t in range(NT):
                units.append(k_unit(t))
            for sub in range(QC // P):
                units.append(v_unit(sub))
            return units

        def outproj_units(oc):
            an_c = attn_by_chunk.pop(oc)
            units = []

            def o_unit(o):
                def f():
                    po = scratch.tile([P, QC], F32, tag="pacc")
                    for t in range(NT):
                        nc.tensor.matmul(
                            po, wo_sb[t][:, o * P:(o + 1) * P], an_c[t],
                            start=(t == 0), stop=(t == NT - 1))
                    ot = otp.tile([P, QC], BF16, tag="ot")
                    nc.scalar.activation(
                        out=ot, in_=po, func=IDENT, bias=bo_sb[:, o:o + 1], scale=1.0)
                    nc.sync.dma_start(
                        out=partTs[oc][o * P:(o + 1) * P, :], in_=ot)
                return f

            for o in range(D // P):
                units.append(o_unit(o))
            return units

        def emit_rs(oc):
            # input deps (partT writers) and completion->consumer sync are
            # wired by the tile framework
            nc.gpsimd.collective_compute(
                "ReduceScatter", mybir.AluOpType.add,
                replica_groups=[[0, 1], [2, 3], [4, 5], [6, 7]],
                ins=[partTs[oc][:, :]],
                outs=[rs_out[oc]],
            )

        # ---------- deferred softmax normalization ----------
        # AV results (attn rows 0..63, Z in row 64) are copied out of PSUM
        # immediately (releasing the accumulator bank for the next pair);
        # the 1/Z multiply runs a pair later, off the critical path.
        norm_fifo = []

        def do_norm(an_t, ua_e, ua_o, z_e, z_o):
            # fast approximate 1/Z (5x cheaper than exact reciprocal); inputs
            # are partition-0-aligned so the custom-DVE op never does a
            # cross-quadrant read
            zr_e = zrp.tile([1, QC], F32, tag="zr")
            zr_o = zrp.tile([1, QC], F32, tag="zr")
            nc.vector.reciprocal_approx_fast(out=zr_e, in_=z_e[0:1, :])
            nc.vector.reciprocal_approx_fast(out=zr_o, in_=z_o[0:1, :])
            bz_e = bzp.tile([DH, QC], F32, tag="bz")
            bz_o = bzp.tile([DH, QC], F32, tag="bz")
            nc.gpsimd.partition_broadcast(bz_e, zr_e[0:1, :], channels=DH)
            nc.gpsimd.partition_broadcast(bz_o, zr_o[0:1, :], channels=DH)
            # the odd head writes partitions 64..127 directly (DVE crossbar)
            nc.vector.tensor_mul(an_t[0:DH, :], ua_e[0:DH, :], bz_e)
            nc.vector.tensor_mul(an_t[DH:P, :], ua_o[0:DH, :], bz_o)

        # ---------- chunk 0 projections up-front ----------
        u0 = proj_units(0)
        u0[0]()      # x tiles for chunk 0 first
        load_wk()
        for u in u0[1:]:
            u()

        # ---------- main loop: attention(c) with interleaved fillers ----------
        for c in range(NQC):
            njt = min(4 * c + 4, NKT)
            ngrp = njt // 2

            fillers = []
            op_u = outproj_units(c - 1) if c > 0 else []
            pr_u = proj_units(c + 1) if c + 1 < NQC else []
            # front-load the output projection (2 o-units per proj unit) so
            # the chunk c-1 partials finish early, then trigger the collective
            # mid-chunk so it overlaps the rest of the attention compute
            i = j = 0
            while i < len(pr_u) or j < len(op_u):
                if j < len(op_u):
                    fillers.append(op_u[j]); j += 1
                if j < len(op_u):
                    fillers.append(op_u[j]); j += 1
                    if j == len(op_u):
                        fillers.append(lambda oc=c - 1: emit_rs(oc))
                if i < len(pr_u):
                    fillers.append(pr_u[i]); i += 1

            nslots = NT * ngrp
            pumped = 0

            # drain all fillers by ~85% of the chunk so the next chunk's
            # attention never waits on this chunk's q/k/v projections
            drain_by = max(1, (nslots * 85) // 100)

            def pump(slot_done):
                nonlocal pumped
                due = (len(fillers) * slot_done + drain_by - 1) // drain_by
                while pumped < min(due, len(fillers)):
                    fillers[pumped]()
                    pumped += 1

            slot = 0
            attn_n = {}
            for t in range(NT):
                an_t = anp.tile([P, QC], BF16, tag="an")
                acc_e = accp.tile([P, QC], F32, tag="acc")
                acc_o = accp.tile([P, QC], F32, tag="acc")
                qt = qT_t[(t, c)]

                def s_group(g):
                    """S^T for k-tiles 2g, 2g+1, both heads packed in PE rows."""
                    st_e = stp.tile([P, 2 * QC], F32, tag="st")
                    st_o = stp.tile([P, 2 * QC], F32, tag="st")
                    for half in range(2):
                        jj = 2 * g + half
                        for par, st in ((0, st_e), (1, st_o)):
                            nc.tensor.matmul(
                                st[:, half * QC:(half + 1) * QC],
                                kT_sb[t][par * DH:(par + 1) * DH,
                                         jj * KT:(jj + 1) * KT],
                                qt[par * DH:(par + 1) * DH, :],
                                start=True, stop=True, skip_group_check=True)
                    return st_e, st_o

                sts = s_group(0)
                pts = {}
                for g in range(ngrp):
                    st_e, st_o = sts
                    if g + 1 < ngrp:
                        sts = s_group(g + 1)
                    # exp
                    pt_e = ptp.tile([P, 2 * QC], BF16, tag="pt")
                    pt_o = ptp.tile([P, 2 * QC], BF16, tag="pt")
                    nc.scalar.activation(out=pt_e, in_=st_e, func=EXP,
                                         scale=float(scale))
                    nc.scalar.activation(out=pt_o, in_=st_o, func=EXP,
                                         scale=float(scale))
                    # causal mask on diagonal k-tiles: only the [128,128]
                    # block at the diagonal needs masking; columns left of it
                    # are excluded by the narrowed AV matmul below.
                    for half in range(2):
                        jj = 2 * g + half
                        sub = jj - 4 * c
                        if 0 <= sub < 4:
                            lo = half * QC + sub * P
                            for pt in (pt_e, pt_o):
                                nc.vector.tensor_mul(
                                    pt[:, lo:lo + P], pt[:, lo:lo + P], tri_sb)
                    pts[g] = (pt_e, pt_o)
                    # AV for k-tiles of this group (narrowed on diagonal)
                    for half in range(2):
                        jj = 2 * g + half
                        sub = jj - 4 * c
                        lo = sub * P if 0 <= sub < 4 else 0
                        for par, acc in ((0, acc_e), (1, acc_o)):
                            h = 2 * t + par
                            pt = pts[g][par]
                            nc.tensor.matmul(
                                acc[0:65, lo:QC],
                                v_sb[jj][:, h, :],
                                pt[:, half * QC + lo:(half + 1) * QC],
                                start=(jj == 0), stop=(jj == njt - 1),
                                skip_group_check=True)
                    if g >= 2:
                        pts.pop(g - 2, None)
                    slot += 1
                    pump(slot)

                # ---- copy attn+Z out of PSUM (ACT takes one half, DVE the
                # other, in parallel), freeing the accumulator banks ----
                ua_e = uap.tile([DH, QC], BF16, tag="ua")
                ua_o = uap.tile([DH, QC], BF16, tag="ua")
                z_e = zvp.tile([1, QC], F32, tag="zv")
                z_o = zvp.tile([1, QC], F32, tag="zv")
                nc.scalar.copy(out=ua_e, in_=acc_e[0:DH, :])
                nc.vector.tensor_copy(out=ua_o, in_=acc_o[0:DH, :])
                nc.vector.tensor_copy(out=z_e, in_=acc_e[64:65, :])
                nc.vector.tensor_copy(out=z_o, in_=acc_o[64:65, :])
                norm_fifo.append((an_t, ua_e, ua_o, z_e, z_o))
                # eager in the last chunk (its out-projection follows directly)
                keep = 0 if c == NQC - 1 else 1
                while len(norm_fifo) > keep:
                    do_norm(*norm_fifo.pop(0))
                attn_n[t] = an_t

            attn_by_chunk[c] = attn_n
            # drain remaining fillers and pending norms
            pump(nslots)
            assert pumped == len(fillers)
            while norm_fifo:
                do_norm(*norm_fifo.pop(0))
            # stream finished chunks to the output while compute continues
            if c >= 2:
                oc = c - 2
                nc.gpsimd.dma_start(
                    out=outTh[oc], in_=rs_out[oc]).then_inc(out_sems[oc], 16)

        # ---------- tail: output projection + RS of the last chunk ----------
        for u in outproj_units(NQC - 1):
            u()
        # chunk NQC-2's output copy runs while partT of the last chunk is
        # still being written; then the last collective and its copy
        nc.gpsimd.dma_start(
            out=outTh[NQC - 2], in_=rs_out[NQC - 2]).then_inc(out_sems[NQC - 2], 16)
        emit_rs(NQC - 1)
        nc.gpsimd.dma_start(
            out=outTh[NQC - 1], in_=rs_out[NQC - 1]).then_inc(out_sems[NQC - 1], 16)
        for oc in range(NQC):
            nc.gpsimd.wait_ge(out_sems[oc], 16)

    nc.compile()
    return nc


def _make_in_maps(x, Wq, bq, Wk, bk, Wv, bv, Wo, bo, mask):
    ref = np.tril(np.ones((L, L), dtype=np.int32))[None, None]
    assert np.array_equal(np.asarray(mask), ref), "mask must be causal"

    # triangular mask for the diagonal [128,128] block: tri[p, f] = 1 if p <= f
    pf = np.arange(P)[None, :] - np.arange(P)[:, None]
    tri = (pf >= 0).astype(ml_dtypes.bfloat16)

    in_maps = []
    for c in range(N_CORES):
        b, g = c // 2, c % 2
        cols = slice(HC * g, HC * g + HC)
        in_maps.append({
            "xT": np.ascontiguousarray(np.asarray(x[b]).T).astype(ml_dtypes.bfloat16),
            "wq": np.ascontiguousarray(np.asarray(Wq)[:, cols]).astype(ml_dtypes.bfloat16),
            "wk": np.ascontiguousarray(np.asarray(Wk)[:, cols]).astype(ml_dtypes.bfloat16),
            "wv": np.ascontiguousarray(np.asarray(Wv)[:, cols]).astype(ml_dtypes.bfloat16),
            "wo": np.ascontiguousarray(np.asarray(Wo)[cols, :]).astype(ml_dtypes.bfloat16),
            "bq": np.ascontiguousarray(np.asarray(bq)[cols].reshape(HC // P, P).T),
            "bk": np.ascontiguousarray(np.asarray(bk)[cols].reshape(HC // P, P).T),
            "bv": np.ascontiguousarray(
                np.broadcast_to(np.asarray(bv)[cols], (P, HC))),
            "bo": np.ascontiguousarray(
                (np.asarray(bo) / 2.0).reshape(D // P, P).T.astype(np.float32)),
            "tri": tri,
        })
    return in_maps


def kernel(x, Wq, bq, Wk, bk, Wv, bv, Wo, bo, mask):
    global _NC, LAST_EXEC_NS
    if _NC is None:
        _NC = build_nc()
    in_maps = _make_in_maps(x, Wq, bq, Wk, bk, Wv, bv, Wo, bo, mask)
    r = run_bass_kernel_spmd(
        _NC, in_maps, core_ids=list(range(N_CORES)), trace=TRACE)
    LAST_EXEC_NS = r.exec_time_ns
    out = np.empty((B, L, D), dtype=np.float32)
    for b in range(B):
        halves = []
        for c in (2 * b, 2 * b + 1):
            arr = np.asarray(r.results[c]["outTh"]).astype(np.float32)
            halves.append(np.concatenate(list(arr), axis=1))  # [D//2, L]
        out[b] = np.concatenate(halves, axis=0).T
    return out


# revision 51
# speedup vs baseline: 1.2132x; 1.0242x over previous
"""Multi-head causal attention on 8 Trainium2 NeuronCores.

Sharding: core c handles batch b = c // 2 and head-group g = c % 2
(8 of 16 heads, i.e. 512 of 1024 projection columns).  QKV projections,
attention and the output projection partial run per-core; the two cores
of a batch pair-ReduceScatter their partial outputs (bf16, chunked over
the sequence and overlapped with compute).

Everything on-device is computed in a transposed layout (seq on the
free dim) so no PE transposes are needed anywhere:
  xT [D, L] (host-pre-transposed, bf16) -> qT/kT [512, L] -> S^T [keys, q]
  -> P^T = exp(S^T) (bf16) -> attn^T = (v|ones)^T @ P^T (Z in row 64)
  -> out^T = Wo^T @ attn_norm^T.  Host un-transposes the result.

Pipelining: the emission order interleaves the projections of chunk c+1
and the output projection of chunk c-1 into the attention groups of
chunk c, so the PE never idles long enough for the HAM clock gate to
re-throttle.  S matmuls for the two heads of a pair run concurrently in
disjoint PE row-groups (tile_position).  Softmax normalization uses
DVE reciprocal -> GpSimd partition_broadcast -> DVE multiply (the odd
head's multiply writes directly to partitions 64..127 via the DVE
output crossbar), with no DRAM round trip.
"""

import sys, types

sys.path.insert(0, "/opt/trn_rl_repo")

# antenv.axon_hooks is missing in this image; inject it so trace=True can
# reach the NTFF profiling hook (used by test.py, off by default).
if "antenv.axon_hooks" not in sys.modules:
    _hook_mod = types.ModuleType("antenv.axon_hooks")
    _hook_mod._hook = None
    def _set_hook(h):
        _hook_mod._hook = h
    def _get_hook():
        return _hook_mod._hook
    _hook_mod.set_axon_ntff_profile_hook = _set_hook
    _hook_mod.get_axon_ntff_profile_hook = _get_hook
    sys.modules["antenv.axon_hooks"] = _hook_mod
    try:
        import antenv
        antenv.axon_hooks = _hook_mod
        from trn_agent_boot.trn_boot import _ntff_profile_via_ctypes
        _set_hook(_ntff_profile_via_ctypes("/opt/axon/libaxon_pjrt.so"))
    except Exception:
        pass

import numpy as np
import ml_dtypes
import concourse.bass as bass
import concourse.mybir as mybir
import concourse.tile as tile
from concourse import bacc
from concourse.bass_utils import run_bass_kernel_spmd

B, L, D, H = 4, 2048, 1024, 16
DH = 64
N_CORES = 8
NH = 8          # heads per core
HC = NH * DH    # 512 projection cols per core
QC = 512        # q-chunk
KT = 128        # k-tile
P = 128

F32 = mybir.dt.float32
BF16 = mybir.dt.bfloat16

TRACE = False
LAST_EXEC_NS = None
_NC = None


def build_nc(seq_len=L):
    Ls = seq_len
    NQC = Ls // QC     # 4 sequence chunks
    NKT = Ls // KT     # 16 key tiles
    NDS = D // P       # 8 contraction tiles for projections
    NT = HC // P       # 4 head pairs
    nc = bacc.Bacc()

    xT = nc.declare_dram_parameter("xT", [D, Ls], BF16, isOutput=False)
    wq = nc.declare_dram_parameter("wq", [D, HC], BF16, isOutput=False)
    wk = nc.declare_dram_parameter("wk", [D, HC], BF16, isOutput=False)
    wv = nc.declare_dram_parameter("wv", [D, HC], BF16, isOutput=False)
    wo = nc.declare_dram_parameter("wo", [HC, D], BF16, isOutput=False)
    bq = nc.declare_dram_parameter("bq", [P, HC // P], F32, isOutput=False)
    bk = nc.declare_dram_parameter("bk", [P, HC // P], F32, isOutput=False)
    bv = nc.declare_dram_parameter("bv", [P, HC], F32, isOutput=False)
    bo = nc.declare_dram_parameter("bo", [P, D // P], F32, isOutput=False)
    tri = nc.declare_dram_parameter("tri", [P, P], BF16, isOutput=False)
    # one contiguous [D//2, QC] block per sequence chunk (collective outputs
    # must be contiguous); the host stitches chunks back together
    outTh = nc.declare_dram_parameter("outTh", [Ls // QC, D // 2, QC], BF16, isOutput=True)

    partTs = [nc.dram_tensor(f"partT{c}", [D, QC], BF16) for c in range(NQC)]
    rs_out = nc.dram_tensor("rs_out", [NQC, D // 2, QC], BF16)

    scale = 1.0 / np.sqrt(np.float32(DH))
    IDENT = mybir.ActivationFunctionType.Identity
    EXP = mybir.ActivationFunctionType.Exp

    from contextlib import ExitStack
    with nc.allow_low_precision(reason="bf16 matmul inputs by design"), \
         tile.TileContext(nc) as tc, ExitStack() as ctx:
        out_sems = [ctx.enter_context(nc.semaphore(f"out_sem{c}"))
                    for c in range(NQC)]
        consts = ctx.enter_context(tc.tile_pool(name="consts", bufs=1))
        wpool = ctx.enter_context(tc.tile_pool(name="wpool", bufs=1))
        kvres = ctx.enter_context(tc.tile_pool(name="kvres", bufs=1))
        xtp = ctx.enter_context(tc.tile_pool(name="xtp", bufs=16))
        qtp = ctx.enter_context(tc.tile_pool(name="qtp", bufs=8))
        ptp = ctx.enter_context(tc.tile_pool(name="ptp", bufs=6))
        anp = ctx.enter_context(tc.tile_pool(name="anp", bufs=8))
        otp = ctx.enter_context(tc.tile_pool(name="otp", bufs=3))
        zrp = ctx.enter_context(tc.tile_pool(name="zrp", bufs=2))
        bzp = ctx.enter_context(tc.tile_pool(name="bzp", bufs=2))
        uap = ctx.enter_context(tc.tile_pool(name="uap", bufs=4))
        zvp = ctx.enter_context(tc.tile_pool(name="zvp", bufs=4))
        scratch = ctx.enter_context(tc.tile_pool(name="scratch", bufs=2, space="PSUM"))
        stp = ctx.enter_context(tc.tile_pool(name="stp", bufs=2, space="PSUM"))
        accp = ctx.enter_context(tc.tile_pool(name="accp", bufs=2, space="PSUM"))

        # ---- constants ----
        bq_sb = consts.tile([P, HC // P], F32, tag="bq")
        bk_sb = consts.tile([P, HC // P], F32, tag="bk")
        bv_sb = consts.tile([P, HC], F32, tag="bv")
        bo_sb = consts.tile([P, D // P], F32, tag="bo")
        tri_sb = consts.tile([P, P], BF16, tag="tri")
        nc.sync.dma_start(out=bq_sb, in_=bq[:, :])
        nc.sync.dma_start(out=bk_sb, in_=bk[:, :])
        nc.sync.dma_start(out=bv_sb, in_=bv[:, :])
        nc.sync.dma_start(out=bo_sb, in_=bo[:, :])
        nc.sync.dma_start(out=tri_sb, in_=tri[:, :])

        # ---- weights resident (bf16); spread across engine DMA queues so
        # the sync queue is free for the chunk-0 x tiles (startup latency) ----
        wq_sb = [wpool.tile([P, HC], BF16, tag=f"wq{ds}", name=f"wq{ds}") for ds in range(NDS)]
        wk_sb = [wpool.tile([P, HC], BF16, tag=f"wk{ds}", name=f"wk{ds}") for ds in range(NDS)]
        wv_sb = [wpool.tile([P, HC], BF16, tag=f"wv{ds}", name=f"wv{ds}") for ds in range(NDS)]
        wo_sb = [wpool.tile([P, D], BF16, tag=f"wo{t}", name=f"wo{t}") for t in range(NT)]
        for ds in range(NDS):
            nc.scalar.dma_start(out=wq_sb[ds], in_=wq[ds * P:(ds + 1) * P, :])
        for ds in range(NDS):
            nc.scalar.dma_start(out=wv_sb[ds], in_=wv[ds * P:(ds + 1) * P, :])
        for t in range(NT):
            nc.gpsimd.dma_start(out=wo_sb[t], in_=wo[t * P:(t + 1) * P, :])

        def load_wk():
            # sync queue (fast HW DGE), after the chunk-0 x tiles
            for ds in range(NDS):
                nc.sync.dma_start(out=wk_sb[ds], in_=wk[ds * P:(ds + 1) * P, :])

        # ---- resident kT and v ----
        kT_sb = [kvres.tile([P, Ls], BF16, tag=f"kT{t}", name=f"kT{t}") for t in range(NT)]
        # v: per key-tile [128, NH, 65] bf16; cols 0..63 = v, col 64 = ones
        # (the ones column makes the AV matmul emit softmax Z in row 64)
        v_sb = [kvres.tile([P, NH, 65], BF16, tag=f"v{kt}", name=f"v{kt}") for kt in range(NKT)]
        for kt in range(NKT):
            nc.vector.memset(v_sb[kt], 1.0)

        xT_t = {}      # (ds, s) -> x tile
        qT_t = {}      # (t, s)  -> q tile
        attn_by_chunk = {}

        # ---------- emission units ----------
        def proj_units(s):
            units = []

            def load_x():
                for ds in range(NDS):
                    xt = xtp.tile([P, QC], BF16, tag="xT")
                    nc.sync.dma_start(
                        out=xt, in_=xT[ds * P:(ds + 1) * P, s * QC:(s + 1) * QC])
                    xT_t[(ds, s)] = xt
            units.append(load_x)

            def q_unit(t):
                def f():
                    pq = scratch.tile([P, QC], F32, tag="pacc")
                    for ds in range(NDS):
                        nc.tensor.matmul(
                            pq, wq_sb[ds][:, t * P:(t + 1) * P], xT_t[(ds, s)],
                            start=(ds == 0), stop=(ds == NDS - 1))
                    qt = qtp.tile([P, QC], BF16, tag="qT")
                    nc.scalar.activation(
                        out=qt, in_=pq, func=IDENT, bias=bq_sb[:, t:t + 1], scale=1.0)
                    qT_t[(t, s)] = qt
                return f

            def k_unit(t):
                def f():
                    pk = scratch.tile([P, QC], F32, tag="pacc")
                    for ds in range(NDS):
                        nc.tensor.matmul(
                            pk, wk_sb[ds][:, t * P:(t + 1) * P], xT_t[(ds, s)],
                            start=(ds == 0), stop=(ds == NDS - 1))
                    nc.scalar.activation(
                        out=kT_sb[t][:, s * QC:(s + 1) * QC], in_=pk,
                        func=IDENT, bias=bk_sb[:, t:t + 1], scale=1.0)
                return f

            def v_unit(sub):
                def f():
                    kt = s * (QC // P) + sub
                    pv = scratch.tile([P, HC], F32, tag="pacc")
                    for ds in range(NDS):
                        nc.tensor.matmul(
                            pv, xT_t[(ds, s)][:, sub * P:(sub + 1) * P], wv_sb[ds],
                            start=(ds == 0), stop=(ds == NDS - 1))
                    nc.vector.tensor_add(
                        v_sb[kt][:, :, 0:64],
                        pv[:].rearrange("p (h d) -> p h d", h=NH),
                        bv_sb[:].rearrange("p (h d) -> p h d", h=NH))
                return f

            for t in range(NT):
                units.append(q_unit(t))
            for t in range(NT):
                units.append(k_unit(t))
            for sub in range(QC // P):
                units.append(v_unit(sub))
            return units

        def outproj_units(oc):
            an_c = attn_by_chunk.pop(oc)
            units = []

            def o_unit(o):
                def f():
                    po = scratch.tile([P, QC], F32, tag="pacc")
                    for t in range(NT):
                        nc.tensor.matmul(
                            po, wo_sb[t][:, o * P:(o + 1) * P], an_c[t],
                            start=(t == 0), stop=(t == NT - 1))
                    ot = otp.tile([P, QC], BF16, tag="ot")
                    nc.scalar.activation(
                        out=ot, in_=po, func=IDENT, bias=bo_sb[:, o:o + 1], scale=1.0)
                    nc.sync.dma_start(
                        out=partTs[oc][o * P:(o + 1) * P, :], in_=ot)
                return f

            for o in range(D // P):
                units.append(o_unit(o))
            return units

        def emit_rs(oc):
            # input deps (partT writers) and completion->consumer sync are
            # wired by the tile framework
            nc.gpsimd.collective_compute(
                "ReduceScatter", mybir.AluOpType.add,
                replica_groups=[[0, 1], [2, 3], [4, 5], [6, 7]],
                ins=[partTs[oc][:, :]],
                outs=[rs_out[oc]],
            )

        # ---------- deferred softmax normalization ----------
        # AV results (attn rows 0..63, Z in row 64) are copied out of PSUM
        # immediately (releasing the accumulator bank for the next pair);
        # the 1/Z multiply runs a pair later, off the critical path.
        norm_fifo = []

        def do_norm(an_t, ua_e, ua_o, z2):
            # fast approximate 1/Z (5x cheaper than exact reciprocal); one
            # fused [1, 2*QC] op + one broadcast for both heads of the pair.
            # Inputs are partition-0-aligned so the custom-DVE op never does
            # a cross-quadrant read.
            zr2 = zrp.tile([1, 2, QC], F32, tag="zr")
            nc.vector.reciprocal_approx_fast(out=zr2, in_=z2[0:1, :, :])
            bz2 = bzp.tile([DH, 2, QC], F32, tag="bz")
            nc.gpsimd.partition_broadcast(bz2, zr2[0:1, :, :], channels=DH)
            # the odd head writes partitions 64..127 directly (DVE crossbar)
            nc.vector.tensor_mul(an_t[0:DH, :], ua_e[0:DH, :], bz2[:, 0, :])
            nc.vector.tensor_mul(an_t[DH:P, :], ua_o[0:DH, :], bz2[:, 1, :])

        # ---------- chunk 0 projections up-front ----------
        u0 = proj_units(0)
        u0[0]()      # x tiles for chunk 0 first
        load_wk()
        for u in u0[1:]:
            u()

        # ---------- main loop: attention(c) with interleaved fillers ----------
        for c in range(NQC):
            njt = min(4 * c + 4, NKT)
            ngrp = njt // 2

            fillers = []
            op_u = outproj_units(c - 1) if c > 0 else []
            pr_u = proj_units(c + 1) if c + 1 < NQC else []
            # front-load the output projection (2 o-units per proj unit) so
            # the chunk c-1 partials finish early, then trigger the collective
            # mid-chunk so it overlaps the rest of the attention compute
            i = j = 0
            while i < len(pr_u) or j < len(op_u):
                if j < len(op_u):
                    fillers.append(op_u[j]); j += 1
                if j < len(op_u):
                    fillers.append(op_u[j]); j += 1
                    if j == len(op_u):
                        fillers.append(lambda oc=c - 1: emit_rs(oc))
                if i < len(pr_u):
                    fillers.append(pr_u[i]); i += 1

            nslots = NT * ngrp
            pumped = 0

            # drain all fillers by ~85% of the chunk so the next chunk's
            # attention never waits on this chunk's q/k/v projections
            drain_by = max(1, (nslots * 85) // 100)

            def pump(slot_done):
                nonlocal pumped
                due = (len(fillers) * slot_done + drain_by - 1) // drain_by
                while pumped < min(due, len(fillers)):
                    fillers[pumped]()
                    pumped += 1

            slot = 0
            attn_n = {}
            for t in range(NT):
                an_t = anp.tile([P, QC], BF16, tag="an")
                acc_e = accp.tile([P, QC], F32, tag="acc")
                acc_o = accp.tile([P, QC], F32, tag="acc")
                qt = qT_t[(t, c)]

                def s_group(g):
                    """S^T for k-tiles 2g, 2g+1, both heads packed in PE rows."""
                    st_e = stp.tile([P, 2 * QC], F32, tag="st")
                    st_o = stp.tile([P, 2 * QC], F32, tag="st")
                    for half in range(2):
                        jj = 2 * g + half
                        for par, st in ((0, st_e), (1, st_o)):
                            nc.tensor.matmul(
                                st[:, half * QC:(half + 1) * QC],
                                kT_sb[t][par * DH:(par + 1) * DH,
                                         jj * KT:(jj + 1) * KT],
                                qt[par * DH:(par + 1) * DH, :],
                                start=True, stop=True, skip_group_check=True)
                    return st_e, st_o

                sts = s_group(0)
                pts = {}
                for g in range(ngrp):
                    st_e, st_o = sts
                    if g + 1 < ngrp:
                        sts = s_group(g + 1)
                    # exp; the last (fully-diagonal) group only computes the
                    # causally-valid column ranges
                    pt_e = ptp.tile([P, 2 * QC], BF16, tag="pt")
                    pt_o = ptp.tile([P, 2 * QC], BF16, tag="pt")
                    if g == 2 * c + 1:
                        for pt, st in ((pt_e, st_e), (pt_o, st_o)):
                            nc.scalar.activation(
                                out=pt[:, 2 * P:QC], in_=st[:, 2 * P:QC],
                                func=EXP, scale=float(scale))
                            nc.scalar.activation(
                                out=pt[:, QC + 3 * P:], in_=st[:, QC + 3 * P:],
                                func=EXP, scale=float(scale))
                    else:
                        nc.scalar.activation(out=pt_e, in_=st_e, func=EXP,
                                             scale=float(scale))
                        nc.scalar.activation(out=pt_o, in_=st_o, func=EXP,
                                             scale=float(scale))
                    # causal mask on diagonal k-tiles: only the [128,128]
                    # block at the diagonal needs masking; columns left of it
                    # are excluded by the narrowed AV matmul below.
                    for half in range(2):
                        jj = 2 * g + half
                        sub = jj - 4 * c
                        if 0 <= sub < 4:
                            lo = half * QC + sub * P
                            for pt in (pt_e, pt_o):
                                nc.vector.tensor_mul(
                                    pt[:, lo:lo + P], pt[:, lo:lo + P], tri_sb)
                    pts[g] = (pt_e, pt_o)
                    # AV for k-tiles of this group (narrowed on diagonal)
                    for half in range(2):
                        jj = 2 * g + half
                        sub = jj - 4 * c
                        lo = sub * P if 0 <= sub < 4 else 0
                        for par, acc in ((0, acc_e), (1, acc_o)):
                            h = 2 * t + par
                            pt = pts[g][par]
                            nc.tensor.matmul(
                                acc[0:65, lo:QC],
                                v_sb[jj][:, h, :],
                                pt[:, half * QC + lo:(half + 1) * QC],
                                start=(jj == 0), stop=(jj == njt - 1),
                                skip_group_check=True)
                    if g >= 2:
                        pts.pop(g - 2, None)
                    slot += 1
                    pump(slot)

                # ---- copy attn+Z out of PSUM (ACT takes one half, DVE the
                # other, in parallel), freeing the accumulator banks ----
                ua_e = uap.tile([DH, QC], BF16, tag="ua")
                ua_o = uap.tile([DH, QC], BF16, tag="ua")
                z2 = zvp.tile([1, 2, QC], F32, tag="zv")
                nc.scalar.copy(out=ua_e, in_=acc_e[0:DH, :])
                nc.vector.tensor_copy(out=ua_o, in_=acc_o[0:DH, :])
                nc.vector.tensor_copy(out=z2[0:1, 0, :], in_=acc_e[64:65, :])
                nc.vector.tensor_copy(out=z2[0:1, 1, :], in_=acc_o[64:65, :])
                norm_fifo.append((an_t, ua_e, ua_o, z2))
                # eager in the last chunk (its out-projection follows directly)
                keep = 0 if c == NQC - 1 else 1
                while len(norm_fifo) > keep:
                    do_norm(*norm_fifo.pop(0))
                attn_n[t] = an_t

            attn_by_chunk[c] = attn_n
            # drain remaining fillers and pending norms
            pump(nslots)
            assert pumped == len(fillers)
            while norm_fifo:
                do_norm(*norm_fifo.pop(0))
            # stream finished chunks to the output while compute continues
            if c >= 2:
                oc = c - 2
                nc.gpsimd.dma_start(
                    out=outTh[oc], in_=rs_out[oc]).then_inc(out_sems[oc], 16)

        # ---------- tail: output projection + RS of the last chunk ----------
        for u in outproj_units(NQC - 1):
            u()
        # chunk NQC-2's output copy runs while partT of the last chunk is
        # still being written; then the last collective and its copy
        nc.gpsimd.dma_start(
            out=outTh[NQC - 2], in_=rs_out[NQC - 2]).then_inc(out_sems[NQC - 2], 16)
        emit_rs(NQC - 1)
        nc.gpsimd.dma_start(
            out=outTh[NQC - 1], in_=rs_out[NQC - 1]).then_inc(out_sems[NQC - 1], 16)
        for oc in range(NQC):
            nc.gpsimd.wait_ge(out_sems[oc], 16)

    nc.compile()
    return nc


def _make_in_maps(x, Wq, bq, Wk, bk, Wv, bv, Wo, bo, mask):
    ref = np.tril(np.ones((L, L), dtype=np.int32))[None, None]
    assert np.array_equal(np.asarray(mask), ref), "mask must be causal"

    # triangular mask for the diagonal [128,128] block: tri[p, f] = 1 if p <= f
    pf = np.arange(P)[None, :] - np.arange(P)[:, None]
    tri = (pf >= 0).astype(ml_dtypes.bfloat16)

    in_maps = []
    for c in range(N_CORES):
        b, g = c // 2, c % 2
        cols = slice(HC * g, HC * g + HC)
        in_maps.append({
            "xT": np.ascontiguousarray(np.asarray(x[b]).T).astype(ml_dtypes.bfloat16),
            "wq": np.ascontiguousarray(np.asarray(Wq)[:, cols]).astype(ml_dtypes.bfloat16),
            "wk": np.ascontiguousarray(np.asarray(Wk)[:, cols]).astype(ml_dtypes.bfloat16),
            "wv": np.ascontiguousarray(np.asarray(Wv)[:, cols]).astype(ml_dtypes.bfloat16),
            "wo": np.ascontiguousarray(np.asarray(Wo)[cols, :]).astype(ml_dtypes.bfloat16),
            "bq": np.ascontiguousarray(np.asarray(bq)[cols].reshape(HC // P, P).T),
            "bk": np.ascontiguousarray(np.asarray(bk)[cols].reshape(HC // P, P).T),
            "bv": np.ascontiguousarray(
                np.broadcast_to(np.asarray(bv)[cols], (P, HC))),
            "bo": np.ascontiguousarray(
                (np.asarray(bo) / 2.0).reshape(D // P, P).T.astype(np.float32)),
            "tri": tri,
        })
    return in_maps


def kernel(x, Wq, bq, Wk, bk, Wv, bv, Wo, bo, mask):
    global _NC, LAST_EXEC_NS
    if _NC is None:
        _NC = build_nc()
    in_maps = _make_in_maps(x, Wq, bq, Wk, bk, Wv, bv, Wo, bo, mask)
    r = run_bass_kernel_spmd(
        _NC, in_maps, core_ids=list(range(N_CORES)), trace=TRACE)
    LAST_EXEC_NS = r.exec_time_ns
    out = np.empty((B, L, D), dtype=np.float32)
    for b in range(B):
        halves = []
        for c in (2 * b, 2 * b + 1):
            arr = np.asarray(r.results[c]["outTh"]).astype(np.float32)
            halves.append(np.concatenate(list(arr), axis=1))  # [D//2, L]
        out[b] = np.concatenate(halves, axis=0).T
    return out
